# revision 22
# baseline (speedup 1.0000x reference)
"""Distributed Trainium2 kernel for gated RoPE attention (2x2048x1024, 16 heads).

Sharding: 8 cores = 2 batches x 4 head-groups (4 heads each).
Host sums the 4 per-batch partials (the tensor-parallel reduce).

v4 = v2's proven inner SDPA (row-tiled score pairs, M=65 PV with the softmax
denominator fused as vaug's 65th column, smh/gsh gating) + a restructured
outer loop:
  - ONE fused SDPA loop over (qt, kc) processing BOTH packed head-pairs per
    iteration -- no warm-keeper, and all background production (k/v tiles,
    later q tiles, gates, gating, out-proj) spreads over all 64 iterations.
  - dc-interleaved prologue: ss partials, q0/k0 projections and v0/v1
    accumulate as each x chunk's DMA lands; first exp at ~15us (was ~48).
  - k tiles stream just-in-time (tile g forced before the kc=4g scores);
    v chunks two iterations ahead; q tiles one q-tile ahead.
  - per-qt gating + output projection drain during the next q-tile.
  - input DMA issue spread over sync/gpsimd/scalar queues; ACT table sets
    (exp, sqrt) preloaded via dummy activations so no table load sits on
    the critical path.
"""

import sys

for _p in ("/opt/trn_rl_repo",):
    if _p not in sys.path:
        sys.path.insert(0, _p)

import numpy as np
import ml_dtypes

import concourse.bass as bass
import concourse.mybir as mybir
import concourse.tile as tile
from concourse import bacc
from concourse.bass_utils import run_bass_kernel_spmd

BF16 = mybir.dt.bfloat16
F32 = mybir.dt.float32
AF = mybir.ActivationFunctionType

DIM = 1024
HEADS = 16
DH = 64
B = 2
N = 2048
NH = 4          # heads per core
NCORES = 8
P = 128
DC = DIM // P   # 8 contraction chunks
QT = 512        # q tile (free dim per matmul)
WQ = 516        # q(256) | k(256) | gates(4)


def build_graph(n=N, dbg=False):
    nc = bacc.Bacc("TRN2", target_bir_lowering=False, debug=False,
                   enable_asserts=False)

    nqt = n // QT       # 4 q tiles
    nkc = n // P        # 16 k chunks

    xT_d = nc.dram_tensor("xT", [DIM, n], BF16, kind="ExternalInput")
    wqkg_d = nc.dram_tensor("w_qkg", [DIM, WQ], BF16, kind="ExternalInput")
    wvp_d = nc.dram_tensor("w_vp", [DIM, NH * 65], BF16, kind="ExternalInput")
    wout_d = nc.dram_tensor("w_out_s", [NH * DH, DIM], BF16, kind="ExternalInput")
    cos_d = nc.dram_tensor("cos_t", [P, n], BF16, kind="ExternalInput")
    sin_d = nc.dram_tensor("sin_t", [P, n], BF16, kind="ExternalInput")
    pswap_d = nc.dram_tensor("pswapT", [P, P], BF16, kind="ExternalInput")
    bgT_d = nc.dram_tensor("bgT", [NH, 1], F32, kind="ExternalInput")
    out_d = nc.dram_tensor("out", [n, DIM], BF16, kind="ExternalOutput")
    if dbg:
        dbg_rstd = nc.dram_tensor("dbg_rstd", [1, n], F32, kind="ExternalOutput")
        dbg_qk = nc.dram_tensor("dbg_qk", [P, 4 * n], BF16, kind="ExternalOutput")
        dbg_g4 = nc.dram_tensor("dbg_g4", [NH, n], F32, kind="ExternalOutput")
        dbg_oTs0 = nc.dram_tensor("dbg_oTs0", [P, n], BF16, kind="ExternalOutput")

    with tile.TileContext(nc) as tc:
        with tc.tile_pool(name="consts", bufs=1) as pc, \
             tc.tile_pool(name="big", bufs=1) as pb, \
             tc.tile_pool(name="work", bufs=2) as pw, \
             tc.tile_pool(name="dram", bufs=1, space="DRAM") as pd, \
             tc.tile_pool(name="probs", bufs=4) as pprob, \
             tc.tile_pool(name="psum", bufs=2, space="PSUM") as ps:

            # ---------------- input DMA (multi-queue issue) ----------------
            # x (4 MB) is the prologue critical path; one queue sustains only
            # ~110 GB/s, so the chunks are spread over the three DMA-capable
            # queues (sync / scalar / gpsimd), interleaved with the other
            # inputs in consumption order.
            xT = pb.tile([P, DC * n], BF16, tag="xT", name="xT")
            cos_t = pc.tile([P, n], BF16, tag="cos", name="cos")
            sin_t = pc.tile([P, n], BF16, tag="sin", name="sin")
            wout = pc.tile([P, 2 * DIM], BF16, tag="wout", name="wout")
            wqkg = pc.tile([P, DC * WQ], BF16, tag="wqkg", name="wqkg")
            wvp = pc.tile([P, DC * NH * 65], BF16, tag="wvp", name="wvp")

            # small constants + ACT table preloads go first on their queues
            onesc = pc.tile([P, 1], BF16, tag="onesc", name="onesc")
            nc.gpsimd.memset(onesc[:], 1.0)
            onesrb = pc.tile([DH + 1, P], BF16, tag="onesrb", name="onesrb")
            nc.gpsimd.memset(onesrb[:], 1.0)
            dum = pc.tile([1, 1], F32, tag="dum", name="dum")
            nc.gpsimd.memset(dum[:], 1.0)
            dum2 = pc.tile([1, 1], F32, tag="dum2", name="dum2")
            nc.scalar.activation(dum2[:], dum[:], AF.Exp)
            nc.scalar.sqrt(dum2[:], dum[:])
            pswap = pc.tile([P, P], BF16, tag="pswap", name="pswap")
            bgT = pc.tile([NH, 1], F32, tag="bgT", name="bgT")

            # all inputs striped round-robin over the 3 DMA-capable queues in
            # strict consumption order (per dc: x chunk, then its weights)
            _dq = [nc.sync, nc.scalar, nc.gpsimd]
            _qi = [0]

            def dq():
                q = _dq[_qi[0] % 3]
                _qi[0] += 1
                return q

            for dc in range(DC):
                dq().dma_start(xT[:, dc * n:(dc + 1) * n],
                               xT_d[dc * P:(dc + 1) * P, :])
                dq().dma_start(wqkg[:, dc * WQ:(dc + 1) * WQ],
                               wqkg_d[dc * P:(dc + 1) * P, :])
                dq().dma_start(wvp[:, dc * NH * 65:(dc + 1) * NH * 65],
                               wvp_d[dc * P:(dc + 1) * P, :])
            for j in range(nqt):
                sl = slice(j * QT, (j + 1) * QT)
                dq().dma_start(cos_t[:, sl], cos_d[:, sl])
                dq().dma_start(sin_t[:, sl], sin_d[:, sl])
                if j == 0:
                    dq().dma_start(pswap[:], pswap_d[:])
                    dq().dma_start(bgT[:], bgT_d[:])
            for ec in range(2):
                dq().dma_start(wout[:, ec * DIM:(ec + 1) * DIM],
                               wout_d[ec * P:(ec + 1) * P, :])

            # ---------------- persistent SBUF ----------------
            qkT = [pb.tile([P, n], BF16, tag=f"qkT{i}", name=f"qkT{i}")
                   for i in range(4)]
            rstd_b = pb.tile([P, n], BF16, tag="rstdb", name="rstdb")
            rstd_p = pb.tile([P, n // P], F32, tag="rstdp", name="rstdp")
            vaug = pb.tile([P, nkc * NH * 65], BF16, tag="vaug", name="vaug")
            oTs = [pb.tile([P, n], BF16, tag=f"oTs{i}", name=f"oTs{i}")
                   for i in range(2)]
            g4 = pb.tile([NH, n], F32, tag="g4", name="g4")
            cosr = pb.tile([P, n], BF16, tag="cosr", name="cosr")
            sinr = pb.tile([P, n], BF16, tag="sinr", name="sinr")
            # packed row-vector tiles: heads 2i / 2i+1 at partitions 0 / 64
            gsh2 = [pb.tile([DH + 1, n], F32, tag=f"gsh{i}", name=f"gsh{i}")
                    for i in range(2)]
            smh2 = [pb.tile([DH + 1, n], F32, tag=f"smh{i}", name=f"smh{i}")
                    for i in range(2)]
            for _t in smh2:
                nc.gpsimd.memset(_t[:], 1.0)

            def gsh(h):
                return gsh2[h // 2][(h % 2) * DH:(h % 2) * DH + 1, :]

            def smh(h):
                return smh2[h // 2][(h % 2) * DH:(h % 2) * DH + 1, :]

            # ================= prologue =================
            # dc-interleaved so the PE consumes each x chunk as its DMA
            # lands: x^2 (DVE) + ss partials, q0/k0 projections (both pairs,
            # psum pair tiles), v chunks 0/1.
            ss2 = [ps.tile([DH + 1, QT], F32, tag="po", name=f"ss{j}")
                   for j in range(2)]
            ppq = ps.tile([P, 2 * QT], F32, tag="sc", name="ppq")
            ppk = ps.tile([P, 2 * QT], F32, tag="sc", name="ppk")
            pv01 = [ps.tile([P, NH * 65], F32, tag="bg", name=f"pv{kc}")
                    for kc in range(2)]
            for dc in range(DC):
                x2 = pw.tile([P, n], BF16, tag="x2", name="x2")
                nc.vector.tensor_mul(x2[:], xT[:, dc * n:(dc + 1) * n],
                                     xT[:, dc * n:(dc + 1) * n])
                st, sp = (dc == 0), (dc == DC - 1)
                for qt in range(nqt):
                    nc.tensor.matmul(
                        ss2[qt // 2][(qt % 2) * DH:(qt % 2) * DH + 1, :],
                        onesc[:], x2[:, qt * QT:(qt + 1) * QT],
                        start=st, stop=sp, skip_group_check=True)
                for et in range(4):
                    pp = ppq if et < 2 else ppk
                    nc.tensor.matmul(
                        pp[:, (et % 2) * QT:(et % 2) * QT + QT],
                        wqkg[:, dc * WQ + et * P:dc * WQ + et * P + P],
                        xT[:, dc * n:dc * n + QT],
                        start=st, stop=sp, skip_group_check=True)
                for kc in range(2):
                    nc.tensor.matmul(
                        pv01[kc][:],
                        xT[:, dc * n + kc * P:dc * n + (kc + 1) * P],
                        wvp[:, dc * NH * 65:(dc + 1) * NH * 65],
                        start=st, stop=sp, skip_group_check=True)
            # drain q0/k0 projections (ACT copies; Copy is in every set)
            for et in range(4):
                pp = ppq if et < 2 else ppk
                nc.scalar.copy(qkT[et][:, 0:QT],
                               pp[:, (et % 2) * QT:(et % 2) * QT + QT])

            # rstd chain, batched wide: sqrt/recip over the whole [65, QT]
            # ss tiles in 2 ops each (junk rows never read downstream)
            rr = []
            for j2 in range(2):
                sq = pw.tile([DH + 1, QT], F32, tag="sq", name="sq")
                nc.scalar.sqrt(sq[:], ss2[j2][:])
                rec = pw.tile([DH + 1, QT], F32, tag="rrec", name="rrec",
                              bufs=2)
                nc.vector.reciprocal_approx_fast(rec[:], sq[:])
                rr.append(rec)
            # reload the exp table now (sqrt done for good)
            nc.scalar.activation(dum2[:], dum[:], AF.Exp)
            # rstd_p [token-partition, chunk] via DRAM round-trip; gates the
            # v drains, so start it before the broadcast work
            scr = pd.tile([1, n], F32, tag="scr", name="scr")
            for qt in range(nqt):
                rb = (qt % 2) * DH
                nc.sync.dma_start(scr[0:1, qt * QT:(qt + 1) * QT],
                                  rr[qt // 2][rb:rb + 1, :])
            nc.sync.dma_start(
                rstd_p[:],
                scr[0:1, :].rearrange("o (c p) -> (o p) c", p=P))
            rr16 = []
            for j2 in range(2):
                r16 = pw.tile([DH + 1, QT], BF16, tag="rr16", name="rr16")
                nc.vector.tensor_copy(r16[:], rr[j2][:])
                rr16.append(r16)
            # broadcast rstd across partitions (PE, K=1, bf16 operands)
            for qt in range(nqt):
                rb = (qt % 2) * DH
                bp = ps.tile([P, QT], F32, tag="po", name="bc")
                nc.tensor.matmul(bp[:], onesrb[rb:rb + 1, :],
                                 rr16[qt // 2][rb:rb + 1, :],
                                 start=True, stop=True, skip_group_check=True)
                nc.vector.tensor_copy(rstd_b[:, qt * QT:(qt + 1) * QT], bp[:])
                sl = slice(qt * QT, (qt + 1) * QT)
                nc.vector.tensor_mul(cosr[:, sl], cos_t[:, sl], rstd_b[:, sl])
                nc.vector.tensor_mul(sinr[:, sl], sin_t[:, sl], rstd_b[:, sl])

            # ---------------- building blocks ----------------
            # QK projection of one 512-token tile (in-loop background).
            def qk_proj_tile(et, j):
                pp = ps.tile([P, QT], F32, tag="bg", name="pp")
                for dc in range(DC):
                    nc.tensor.matmul(
                        pp[:],
                        wqkg[:, dc * WQ + et * P:dc * WQ + et * P + P],
                        xT[:, dc * n + j * QT:dc * n + (j + 1) * QT],
                        start=(dc == 0), stop=(dc == DC - 1),
                        skip_group_check=True)
                    yield 216
                nc.vector.tensor_copy(qkT[et][:, j * QT:(j + 1) * QT], pp[:])
                yield 0

            # RoPE on one 512 slice of one packed tile, in place
            # (quadrant-tiled pswap); cosr/sinr carry the rstd token scale.
            def rope_piece(et, j):
                sl = slice(j * QT, (j + 1) * QT)
                t1 = pw.tile([P, QT], BF16, tag="ropec", name="t1")
                nc.vector.tensor_mul(t1[:], qkT[et][:, sl], cosr[:, sl])
                qks = pw.tile([P, QT], BF16, tag="ropes", name="qks")
                nc.vector.tensor_mul(qks[:], qkT[et][:, sl], sinr[:, sl])
                pr = ps.tile([P, QT], F32, tag="bg", name="pr")
                nc.tensor.matmul(pr[0:DH, :], pswap[0:DH, 0:DH],
                                 qks[0:DH, :], start=True, stop=True,
                                 skip_group_check=True)
                nc.tensor.matmul(pr[DH:P, :], pswap[DH:P, DH:P],
                                 qks[DH:P, :], start=True, stop=True,
                                 skip_group_check=True)
                yield 230
                nc.vector.tensor_add(qkT[et][:, sl], t1[:], pr[:])
                yield 0

            # V projection for one k-chunk, all 4 heads at once.
            def v_chunk(kc):
                pv = ps.tile([P, NH * 65], F32, tag="bg", name="pv")
                for dc in range(DC):
                    nc.tensor.matmul(
                        pv[:],
                        xT[:, dc * n + kc * P:dc * n + (kc + 1) * P],
                        wvp[:, dc * NH * 65:(dc + 1) * NH * 65],
                        start=(dc == 0), stop=(dc == DC - 1),
                        skip_group_check=True)
                vsl = slice(kc * NH * 65, (kc + 1) * NH * 65)
                nc.vector.tensor_scalar_mul(vaug[:, vsl], pv[:],
                                            rstd_p[:, kc:kc + 1])
                nc.gpsimd.memset(vaug[:, kc * NH * 65 + DH::65], 1.0)

            def drain_v01():
                for kc in range(2):
                    vsl = slice(kc * NH * 65, (kc + 1) * NH * 65)
                    nc.vector.tensor_scalar_mul(vaug[:, vsl], pv01[kc][:],
                                                rstd_p[:, kc:kc + 1])
                    nc.gpsimd.memset(vaug[:, kc * NH * 65 + DH::65], 1.0)

            # gates: col-tiled M=4 matmuls, all 4 heads at once; sigmoid as
            # 0.5*tanh(z/2)+0.5 (tanh shares the exp ACT table; bgT holds
            # b_gates/2 host-side).
            def gates_proj():
                pg2 = [ps.tile([DH + NH, QT], F32, tag="bg", name=f"pg{j}")
                       for j in range(2)]
                for dc in range(DC):
                    for qt in range(nqt):
                        rb = (qt % 2) * DH
                        nc.tensor.matmul(
                            pg2[qt // 2][rb:rb + NH, :],
                            wqkg[:, dc * WQ + 512:dc * WQ + 516],
                            xT[:, dc * n + qt * QT:dc * n + (qt + 1) * QT],
                            start=(dc == 0), stop=(dc == DC - 1),
                            skip_group_check=True)
                    yield 250
                for qt in range(nqt):
                    sl = slice(qt * QT, (qt + 1) * QT)
                    rb = (qt % 2) * DH
                    nc.vector.tensor_mul(g4[0:NH, sl],
                                         pg2[qt // 2][rb:rb + NH, :],
                                         rstd_b[rb:rb + NH, sl])
                    yield 0
                nc.scalar.activation(g4[:], g4[:], AF.Tanh, scale=0.5,
                                     bias=bgT[:])
                nc.vector.tensor_scalar(g4[:], g4[:], 0.5, 0.5,
                                        mybir.AluOpType.mult,
                                        mybir.AluOpType.add)
                yield 0
                # scatter head rows into the packed gsh2 tiles via DRAM
                scr4 = pd.tile([NH, n], F32, tag="scr4", name="scr4")
                nc.sync.dma_start(scr4[:], g4[:])
                for h in range(NH):
                    nc.sync.dma_start(gsh(h), scr4[h:h + 1, :])
                yield 0
                if dbg:
                    nc.sync.dma_start(dbg_g4[:], g4[:])
                    yield 0

            # gating of one head pair's finished 512-slice + its out chunks
            def gate_slice(i, qt):
                qsl = slice(qt * QT, (qt + 1) * QT)
                rec = pw.tile([DH + 1, QT], F32, tag="recs", name="rec")
                nc.vector.reciprocal_approx_fast(rec[:], smh2[i][:, qsl])
                yield 0
                for h in (2 * i, 2 * i + 1):
                    rb = (h % 2) * DH
                    ft = pw.tile([1, QT], BF16, tag="fts", name="ft")
                    nc.vector.tensor_mul(ft[:], rec[rb:rb + 1, :],
                                         gsh2[i][rb:rb + 1, qsl])
                    pf = ps.tile([DH, QT], F32, tag="bg", name="pf")
                    nc.tensor.matmul(pf[:], onesrb[0:1, 0:DH], ft[0:1, :],
                                     start=True, stop=True)
                    nc.vector.tensor_mul(oTs[i][rb:rb + DH, qsl],
                                         oTs[i][rb:rb + DH, qsl], pf[:])
                    yield 213

            # one n-chunk of the output projection, DMA'd out when built
            # (DMA queue rotated so the tail chunks transfer in parallel)
            def out_nt(nt):
                ob = pw.tile([P, DIM], BF16, tag="ob", name="ob")
                for dh in range(2):
                    pp2 = ps.tile([P, QT], F32, tag="bg", name="pp2")
                    for ec in range(2):
                        nc.tensor.matmul(
                            pp2[:],
                            oTs[ec][:, nt * P:(nt + 1) * P],
                            wout[:, ec * DIM + dh * QT:
                                 ec * DIM + dh * QT + QT],
                            start=(ec == 0), stop=(ec == 1))
                        yield 216
                    if dh == 0:
                        nc.vector.tensor_copy(ob[:, dh * QT:(dh + 1) * QT],
                                              pp2[:])
                    else:
                        nc.scalar.copy(ob[:, dh * QT:(dh + 1) * QT], pp2[:])
                    yield 0
                _dq[nt % 3].dma_start(out_d[nt * P:(nt + 1) * P, :], ob[:])
                yield 0

            def run_now(gen):
                for _ in gen:
                    pass

            def chain(*gens):
                for g in gens:
                    for c in g:
                        yield c

            # rope on the prologue tiles; v0/v1 drains (wait on rstd_p)
            run_now(rope_piece(0, 0))
            run_now(rope_piece(1, 0))
            run_now(rope_piece(2, 0))
            run_now(rope_piece(3, 0))
            drain_v01()

            if dbg:
                nc.sync.dma_start(dbg_rstd[:], scr[:])

            # ---------------- fused SDPA ----------------
            state = {"credit": 0, "gens": []}

            def drain(budget):
                state["credit"] += budget
                gens = state["gens"]
                while gens and state["credit"] > 0:
                    try:
                        state["credit"] -= next(gens[0])
                    except StopIteration:
                        gens.pop(0)

            def force(gen):
                run_now(gen)

            def pv65(pr, kc, pos, pt):
                voff = pt * 2 * 65
                for e, prs in enumerate((pr[:, 0:QT], pr[:, QT:2 * QT])):
                    nc.tensor.matmul(
                        pos[e][:],
                        vaug[:, kc * NH * 65 + voff + e * 65:
                             kc * NH * 65 + voff + (e + 1) * 65],
                        prs,
                        start=(kc == 0), stop=(kc == nkc - 1),
                        skip_group_check=True)

            # k / q production units: et 2+pt is the k tile, et pt the q tile
            ktile = {(pt, g): chain(qk_proj_tile(2 + pt, g),
                                    rope_piece(2 + pt, g))
                     for pt in range(2) for g in range(1, nqt)}
            qtile = {(pt, j): chain(qk_proj_tile(pt, j), rope_piece(pt, j))
                     for pt in range(2) for j in range(1, nqt)}
            g_gates = gates_proj()

            # 8 segments: (qt, pt) with pt inner.  Background inventory is
            # spread so each segment's ACT exp stream stays dense.
            for qt in range(nqt):
                qsl = slice(qt * QT, (qt + 1) * QT)
                for pt in range(2):
                    seg0 = (qt == 0 and pt == 0)
                    pos = [ps.tile([DH + 1, QT], F32, tag="po", name=f"po{e}")
                           for e in range(2)]

                    if seg0:
                        state["gens"] = [ktile[(0, 1)], ktile[(0, 2)],
                                         ktile[(0, 3)], ktile[(1, 1)],
                                         ktile[(1, 2)], ktile[(1, 3)],
                                         g_gates]
                    elif qt == 0 and pt == 1:
                        state["gens"].append(qtile[(0, 1)])
                        state["gens"].append(qtile[(1, 1)])
                    else:
                        if pt == 0:
                            # pt0 of the previous q-tile was gated one
                            # segment ago (qt >= 2) or now (qt == 1, since
                            # gates land at the end of segment 1)
                            if qt == 1:
                                state["gens"].append(chain(
                                    gate_slice(0, 0), gate_slice(1, 0),
                                    *[out_nt(nt) for nt in range(0, 4)]))
                            else:
                                state["gens"].append(chain(
                                    gate_slice(1, qt - 1),
                                    *[out_nt(nt) for nt in
                                      range(4 * (qt - 1), 4 * (qt - 1) + 4)]))
                        else:
                            # this q-tile's pt0 heads are final: gate them now
                            state["gens"].append(gate_slice(0, qt))
                            if qt + 1 < nqt:
                                state["gens"].append(qtile[(0, qt + 1)])
                                state["gens"].append(qtile[(1, qt + 1)])

                    prev = None
                    for kc in range(nkc):
                        ksl = slice(kc * P, (kc + 1) * P)
                        if qt == 0 and kc >= 4 and kc % 4 == 0:
                            # emission-order deadline: k-tile g fully
                            # emitted before the first scores that read it
                            force(ktile[(pt, kc // 4)])
                        if prev is not None:
                            pv65(*prev, pos, pt)
                        if seg0:
                            if kc + 2 < nkc:
                                v_chunk(kc + 2)
                            drain(200)
                        else:
                            drain(500)
                        sc = ps.tile([P, 2 * QT], F32, tag="sc", name="sc")
                        nc.tensor.matmul(sc[:, 0:QT], qkT[2 + pt][0:DH, ksl],
                                         qkT[pt][0:DH, qsl],
                                         start=True, stop=True,
                                         skip_group_check=True)
                        nc.tensor.matmul(sc[:, QT:2 * QT],
                                         qkT[2 + pt][DH:P, ksl],
                                         qkT[pt][DH:P, qsl],
                                         start=True, stop=True,
                                         skip_group_check=True)
                        pr = pprob.tile([P, 2 * QT], BF16, tag="pr", name="pr")
                        nc.scalar.activation(pr[:], sc[:], AF.Exp,
                                             scale=float(DH) ** -0.5)
                        prev = (pr, kc)
                    pv65(*prev, pos, pt)

                    # epilogue: drain finished pos tiles into oTs/smh
                    for e in range(2):
                        h, rb = 2 * pt + e, e * DH
                        nc.vector.tensor_copy(oTs[pt][rb:rb + DH, qsl],
                                              pos[e][0:DH, :])
                        nc.scalar.copy(smh(h)[0:1, qsl],
                                       pos[e][DH:DH + 1, :])
                    if qt == 0 and pt == 1:
                        # gates fully emitted before gate_slice(., 0)
                        force(g_gates)
                    if pt == 1 and qt + 1 < nqt:
                        # q(qt+1) fully emitted before the next scores
                        force(qtile[(0, qt + 1)])
                        force(qtile[(1, qt + 1)])

            # tail: flush remaining background (includes gate_slice(0, 3)),
            # then the last q-tile's pt1 gating + output projection
            for g in state["gens"]:
                run_now(g)
            state["gens"] = []
            run_now(chain(gate_slice(1, nqt - 1),
                          *[out_nt(nt) for nt in range(4 * (nqt - 1), 4 * nqt)]))

            if dbg:
                nc.sync.dma_start(dbg_qk[:, 0:n], qkT[0][:])
                nc.sync.dma_start(dbg_qk[:, n:2 * n], qkT[1][:])
                nc.sync.dma_start(dbg_qk[:, 2 * n:3 * n], qkT[2][:])
                nc.sync.dma_start(dbg_qk[:, 3 * n:4 * n], qkT[3][:])
                nc.sync.dma_start(dbg_oTs0[:], oTs[0][:])

    nc.compile()
    return nc


def host_prep(x, gamma, w_qkv, w_gates, b_gates, w_out, freqs, n=N):
    """Build the 8 per-core input maps (numpy, host-side)."""
    x = np.asarray(x, dtype=np.float32)
    gamma = np.asarray(gamma, dtype=np.float32)
    w_qkv = np.asarray(w_qkv, dtype=np.float32)
    w_gates = np.asarray(w_gates, dtype=np.float32)
    b_gates = np.asarray(b_gates, dtype=np.float32)
    w_out = np.asarray(w_out, dtype=np.float32)
    freqs = np.asarray(freqs, dtype=np.float32)

    bf = ml_dtypes.bfloat16
    gvec = gamma * (DIM ** 0.5)

    pos = np.arange(n, dtype=np.float32)
    ang = pos[:, None] * freqs[None, :]          # [n, 32]
    idx = (np.arange(P) % DH) // 2               # row -> freq index
    cos_t = np.cos(ang)[:, idx].T.astype(bf)     # [128, n]
    sin_t = np.sin(ang)[:, idx].T.astype(bf)

    PT = np.zeros((DH, DH), dtype=np.float32)
    for i in range(DH // 2):
        PT[2 * i + 1, 2 * i] = -1.0
        PT[2 * i, 2 * i + 1] = 1.0
    pswapT = np.zeros((P, P), dtype=np.float32)
    pswapT[0:DH, 0:DH] = PT
    pswapT[DH:P, DH:P] = PT
    pswapT = pswapT.astype(bf)

    in_maps = []
    for c in range(NCORES):
        bi, hg = divmod(c, 4)
        hs = hg * NH
        xT = np.ascontiguousarray(x[bi, :n].T).astype(bf)
        wq = w_qkv[:, hs * DH:(hs + NH) * DH]
        wk = w_qkv[:, HEADS * DH + hs * DH:HEADS * DH + (hs + NH) * DH]
        wv = w_qkv[:, 2 * HEADS * DH + hs * DH:2 * HEADS * DH + (hs + NH) * DH]
        wg = w_gates[:, hs:hs + NH]
        w_qkg = (np.concatenate([wq, wk, wg], axis=1)
                 * gvec[:, None]).astype(bf)
        w_vp = np.zeros((DIM, NH * 65), dtype=np.float32)
        for h in range(NH):
            w_vp[:, h * 65:h * 65 + DH] = wv[:, h * DH:(h + 1) * DH]
        w_vp = (w_vp * gvec[:, None]).astype(bf)
        w_out_s = w_out[hs * DH:(hs + NH) * DH, :].astype(bf)
        # halved: the kernel computes sigmoid(z+b) as 0.5*tanh((z+b)/2)+0.5
        bgT = (b_gates[hs:hs + NH] / 2.0).reshape(NH, 1).astype(np.float32)
        in_maps.append({
            "xT": xT, "w_qkg": w_qkg, "w_vp": w_vp, "w_out_s": w_out_s,
            "cos_t": cos_t, "sin_t": sin_t, "pswapT": pswapT,
            "bgT": bgT,
        })
    return in_maps


_NC_CACHE = {}


def _ensure_ntff_hook():
    """antenv.axon_hooks is missing on this image; recreate it and register
    the ctypes NTFF profiling hook from trn_agent_boot so trace=True works."""
    try:
        from antenv.axon_hooks import get_axon_ntff_profile_hook  # noqa: F401
        return
    except ImportError:
        pass
    import types
    try:
        import antenv
    except ImportError:
        return
    mod = types.ModuleType("antenv.axon_hooks")
    holder = {}
    mod.set_axon_ntff_profile_hook = lambda h: holder.__setitem__("h", h)
    mod.get_axon_ntff_profile_hook = lambda: holder.get("h")
    sys.modules["antenv.axon_hooks"] = mod
    antenv.axon_hooks = mod
    try:
        from trn_agent_boot.trn_boot import _ntff_profile_via_ctypes
        h = _ntff_profile_via_ctypes("/opt/axon/libaxon_pjrt.so")
        if h is not None:
            mod.set_axon_ntff_profile_hook(h)
    except Exception:
        pass


def run(inputs, trace=False, n=N, dbg=False):
    if trace:
        _ensure_ntff_hook()
    key = (n, dbg)
    if key not in _NC_CACHE:
        _NC_CACHE[key] = build_graph(n, dbg=dbg)
    nc = _NC_CACHE[key]
    in_maps = host_prep(**inputs, n=n)
    kw = {}
    if trace:
        kw = dict(trace=True, trace_cores=[0])
    res = run_bass_kernel_spmd(nc, in_maps, core_ids=list(range(NCORES)), **kw)
    parts = [np.asarray(r["out"], dtype=np.float32) for r in res.results]
    out = np.stack([
        parts[0] + parts[1] + parts[2] + parts[3],
        parts[4] + parts[5] + parts[6] + parts[7],
    ]).astype(np.float32)
    return out, res


def kernel(**inputs):
    out, _ = run(inputs, trace=False)
    return out


# revision 24
# speedup vs baseline: 1.1926x; 1.1926x over previous
"""Distributed Trainium2 kernel for gated RoPE attention (2x2048x1024, 16 heads).

Sharding: 8 cores = 2 batches x 4 head-groups (4 heads each).
Host sums the 4 per-batch partials (the tensor-parallel reduce).

v4 = v2's proven inner SDPA (row-tiled score pairs, M=65 PV with the softmax
denominator fused as vaug's 65th column, smh/gsh gating) + a restructured
outer loop:
  - ONE fused SDPA loop over (qt, kc) processing BOTH packed head-pairs per
    iteration -- no warm-keeper, and all background production (k/v tiles,
    later q tiles, gates, gating, out-proj) spreads over all 64 iterations.
  - dc-interleaved prologue: ss partials, q0/k0 projections and v0/v1
    accumulate as each x chunk's DMA lands; first exp at ~15us (was ~48).
  - k tiles stream just-in-time (tile g forced before the kc=4g scores);
    v chunks two iterations ahead; q tiles one q-tile ahead.
  - per-qt gating + output projection drain during the next q-tile.
  - input DMA issue spread over sync/gpsimd/scalar queues; ACT table sets
    (exp, sqrt) preloaded via dummy activations so no table load sits on
    the critical path.
"""

import sys

for _p in ("/opt/trn_rl_repo",):
    if _p not in sys.path:
        sys.path.insert(0, _p)

import numpy as np
import ml_dtypes

import concourse.bass as bass
import concourse.mybir as mybir
import concourse.tile as tile
from concourse import bacc
from concourse.bass_utils import run_bass_kernel_spmd

BF16 = mybir.dt.bfloat16
F32 = mybir.dt.float32
AF = mybir.ActivationFunctionType

DIM = 1024
HEADS = 16
DH = 64
B = 2
N = 2048
NH = 4          # heads per core
NCORES = 8
P = 128
DC = DIM // P   # 8 contraction chunks
QT = 512        # q tile (free dim per matmul)
WQ = 516        # q(256) | k(256) | gates(4)


def build_graph(n=N, dbg=False):
    nc = bacc.Bacc("TRN2", target_bir_lowering=False, debug=False,
                   enable_asserts=False)

    nqt = n // QT       # 4 q tiles
    nkc = n // P        # 16 k chunks

    xT_d = nc.dram_tensor("xT", [DIM, n], BF16, kind="ExternalInput")
    wqkg_d = nc.dram_tensor("w_qkg", [DIM, WQ], BF16, kind="ExternalInput")
    wvp_d = nc.dram_tensor("w_vp", [DIM, NH * 65], BF16, kind="ExternalInput")
    wout_d = nc.dram_tensor("w_out_s", [NH * DH, DIM], BF16, kind="ExternalInput")
    cos_d = nc.dram_tensor("cos_t", [P, n], BF16, kind="ExternalInput")
    sin_d = nc.dram_tensor("sin_t", [P, n], BF16, kind="ExternalInput")
    pswap_d = nc.dram_tensor("pswapT", [P, P], BF16, kind="ExternalInput")
    bgT_d = nc.dram_tensor("bgT", [NH, 1], F32, kind="ExternalInput")
    out_d = nc.dram_tensor("out", [n, DIM], BF16, kind="ExternalOutput")
    if dbg:
        dbg_rstd = nc.dram_tensor("dbg_rstd", [1, n], F32, kind="ExternalOutput")
        dbg_qk = nc.dram_tensor("dbg_qk", [P, 4 * n], BF16, kind="ExternalOutput")
        dbg_g4 = nc.dram_tensor("dbg_g4", [NH, n], F32, kind="ExternalOutput")
        dbg_oTs0 = nc.dram_tensor("dbg_oTs0", [P, n], BF16, kind="ExternalOutput")

    with tile.TileContext(nc) as tc:
        with tc.tile_pool(name="consts", bufs=1) as pc, \
             tc.tile_pool(name="big", bufs=1) as pb, \
             tc.tile_pool(name="work", bufs=2) as pw, \
             tc.tile_pool(name="dram", bufs=1, space="DRAM") as pd, \
             tc.tile_pool(name="probs", bufs=4) as pprob, \
             tc.tile_pool(name="psum", bufs=2, space="PSUM") as ps:

            # ---------------- input DMA (multi-queue issue) ----------------
            # x (4 MB) is the prologue critical path; one queue sustains only
            # ~110 GB/s, so the chunks are spread over the three DMA-capable
            # queues (sync / scalar / gpsimd), interleaved with the other
            # inputs in consumption order.
            xT = pb.tile([P, DC * n], BF16, tag="xT", name="xT")
            cos_t = pc.tile([P, n], BF16, tag="cos", name="cos")
            sin_t = pc.tile([P, n], BF16, tag="sin", name="sin")
            wout = pc.tile([P, 2 * DIM], BF16, tag="wout", name="wout")
            wqkg = pc.tile([P, DC * WQ], BF16, tag="wqkg", name="wqkg")
            wvp = pc.tile([P, DC * NH * 65], BF16, tag="wvp", name="wvp")

            # small constants + ACT table preloads go first on their queues
            onesc = pc.tile([P, 1], BF16, tag="onesc", name="onesc")
            nc.gpsimd.memset(onesc[:], 1.0)
            onesrb = pc.tile([DH + 1, P], BF16, tag="onesrb", name="onesrb")
            nc.gpsimd.memset(onesrb[:], 1.0)
            dum = pc.tile([1, 1], F32, tag="dum", name="dum")
            nc.gpsimd.memset(dum[:], 1.0)
            dum2 = pc.tile([1, 1], F32, tag="dum2", name="dum2")
            nc.scalar.activation(dum2[:], dum[:], AF.Exp)
            nc.scalar.sqrt(dum2[:], dum[:])
            pswap = pc.tile([P, P], BF16, tag="pswap", name="pswap")
            bgT = pc.tile([NH, 1], F32, tag="bgT", name="bgT")

            # all inputs striped round-robin over the 3 DMA-capable queues in
            # strict consumption order (per dc: x chunk, then its weights)
            _dq = [nc.sync, nc.scalar, nc.gpsimd]
            _qi = [0]

            def dq():
                q = _dq[_qi[0] % 3]
                _qi[0] += 1
                return q

            for dc in range(DC):
                _dq[dc % 3].dma_start(xT[:, dc * n:(dc + 1) * n],
                                      xT_d[dc * P:(dc + 1) * P, :])
                _dq[(dc + 1) % 3].dma_start(wqkg[:, dc * WQ:(dc + 1) * WQ],
                                            wqkg_d[dc * P:(dc + 1) * P, :])
                _dq[(dc + 2) % 3].dma_start(
                    wvp[:, dc * NH * 65:(dc + 1) * NH * 65],
                    wvp_d[dc * P:(dc + 1) * P, :])
                _qi[0] += 3
            for j in range(nqt):
                sl = slice(j * QT, (j + 1) * QT)
                dq().dma_start(cos_t[:, sl], cos_d[:, sl])
                dq().dma_start(sin_t[:, sl], sin_d[:, sl])
                if j == 0:
                    dq().dma_start(pswap[:], pswap_d[:])
                    dq().dma_start(bgT[:], bgT_d[:])
            for ec in range(2):
                dq().dma_start(wout[:, ec * DIM:(ec + 1) * DIM],
                               wout_d[ec * P:(ec + 1) * P, :])

            # ---------------- persistent SBUF ----------------
            qkT = [pb.tile([P, n], BF16, tag=f"qkT{i}", name=f"qkT{i}")
                   for i in range(4)]
            rstd_b = pb.tile([P, n], BF16, tag="rstdb", name="rstdb")
            rstd_p = pb.tile([P, n // P], F32, tag="rstdp", name="rstdp")
            vaug = pb.tile([P, nkc * NH * 65], BF16, tag="vaug", name="vaug")
            oTs = [pb.tile([P, n], BF16, tag=f"oTs{i}", name=f"oTs{i}")
                   for i in range(2)]
            g4 = pb.tile([NH, n], F32, tag="g4", name="g4")
            cosr = pb.tile([P, n], BF16, tag="cosr", name="cosr")
            sinr = pb.tile([P, n], BF16, tag="sinr", name="sinr")
            # packed row-vector tiles: heads 2i / 2i+1 at partitions 0 / 64
            gsh2 = [pb.tile([DH + 1, n], F32, tag=f"gsh{i}", name=f"gsh{i}")
                    for i in range(2)]
            smh2 = [pb.tile([DH + 1, n], F32, tag=f"smh{i}", name=f"smh{i}")
                    for i in range(2)]
            for _t in smh2:
                nc.gpsimd.memset(_t[:], 1.0)

            def gsh(h):
                return gsh2[h // 2][(h % 2) * DH:(h % 2) * DH + 1, :]

            def smh(h):
                return smh2[h // 2][(h % 2) * DH:(h % 2) * DH + 1, :]

            # ================= prologue =================
            # dc-interleaved so the PE consumes each x chunk as its DMA
            # lands: x^2 (DVE) + ss partials, q0/k0 projections (both pairs,
            # psum pair tiles), v chunks 0/1.
            ss2 = [ps.tile([DH + 1, QT], F32, tag="po", name=f"ss{j}")
                   for j in range(2)]
            ppq = ps.tile([P, 2 * QT], F32, tag="sc", name="ppq")
            ppk = ps.tile([P, 2 * QT], F32, tag="sc", name="ppk")
            pv01 = [ps.tile([P, NH * 65], F32, tag="bg", name=f"pv{kc}")
                    for kc in range(2)]
            for dc in range(DC):
                x2 = pw.tile([P, n], BF16, tag="x2", name="x2")
                nc.vector.tensor_mul(x2[:], xT[:, dc * n:(dc + 1) * n],
                                     xT[:, dc * n:(dc + 1) * n])
                st, sp = (dc == 0), (dc == DC - 1)
                for qt in range(nqt):
                    nc.tensor.matmul(
                        ss2[qt // 2][(qt % 2) * DH:(qt % 2) * DH + 1, :],
                        onesc[:], x2[:, qt * QT:(qt + 1) * QT],
                        start=st, stop=sp, skip_group_check=True)
                for et in range(4):
                    pp = ppq if et < 2 else ppk
                    nc.tensor.matmul(
                        pp[:, (et % 2) * QT:(et % 2) * QT + QT],
                        wqkg[:, dc * WQ + et * P:dc * WQ + et * P + P],
                        xT[:, dc * n:dc * n + QT],
                        start=st, stop=sp, skip_group_check=True)
                for kc in range(2):
                    nc.tensor.matmul(
                        pv01[kc][:],
                        xT[:, dc * n + kc * P:dc * n + (kc + 1) * P],
                        wvp[:, dc * NH * 65:(dc + 1) * NH * 65],
                        start=st, stop=sp, skip_group_check=True)
            # drain q0/k0 projections (ACT copies; Copy is in every set)
            for et in range(4):
                pp = ppq if et < 2 else ppk
                nc.scalar.copy(qkT[et][:, 0:QT],
                               pp[:, (et % 2) * QT:(et % 2) * QT + QT])

            # rstd chain, batched wide: sqrt/recip over the whole [65, QT]
            # ss tiles in 2 ops each (junk rows never read downstream)
            rr = []
            for j2 in range(2):
                sq = pw.tile([DH + 1, QT], F32, tag="sq", name="sq")
                nc.scalar.sqrt(sq[:], ss2[j2][:])
                rec = pw.tile([DH + 1, QT], F32, tag="rrec", name="rrec",
                              bufs=2)
                nc.vector.reciprocal_approx_fast(rec[:], sq[:])
                rr.append(rec)
            # reload the exp table now (sqrt done for good)
            nc.scalar.activation(dum2[:], dum[:], AF.Exp)
            # rstd_p [token-partition, chunk] via DRAM round-trip; gates the
            # v drains, so start it before the broadcast work
            scr = pd.tile([1, n], F32, tag="scr", name="scr")
            for qt in range(nqt):
                rb = (qt % 2) * DH
                nc.sync.dma_start(scr[0:1, qt * QT:(qt + 1) * QT],
                                  rr[qt // 2][rb:rb + 1, :])
            nc.sync.dma_start(
                rstd_p[:],
                scr[0:1, :].rearrange("o (c p) -> (o p) c", p=P))
            rr16 = []
            for j2 in range(2):
                r16 = pw.tile([DH + 1, QT], BF16, tag="rr16", name="rr16")
                nc.vector.tensor_copy(r16[:], rr[j2][:])
                rr16.append(r16)
            # broadcast rstd across partitions (PE, K=1, bf16 operands)
            for qt in range(nqt):
                rb = (qt % 2) * DH
                bp = ps.tile([P, QT], F32, tag="po", name="bc")
                nc.tensor.matmul(bp[:], onesrb[rb:rb + 1, :],
                                 rr16[qt // 2][rb:rb + 1, :],
                                 start=True, stop=True, skip_group_check=True)
                nc.vector.tensor_copy(rstd_b[:, qt * QT:(qt + 1) * QT], bp[:])
                sl = slice(qt * QT, (qt + 1) * QT)
                nc.vector.tensor_mul(cosr[:, sl], cos_t[:, sl], rstd_b[:, sl])
                nc.vector.tensor_mul(sinr[:, sl], sin_t[:, sl], rstd_b[:, sl])

            # ---------------- building blocks ----------------
            # QK projection of one 512-token tile (in-loop background).
            def qk_proj_tile(et, j):
                pp = ps.tile([P, QT], F32, tag="bg", name="pp")
                for dc in range(DC):
                    nc.tensor.matmul(
                        pp[:],
                        wqkg[:, dc * WQ + et * P:dc * WQ + et * P + P],
                        xT[:, dc * n + j * QT:dc * n + (j + 1) * QT],
                        start=(dc == 0), stop=(dc == DC - 1),
                        skip_group_check=True)
                    yield 216
                nc.vector.tensor_copy(qkT[et][:, j * QT:(j + 1) * QT], pp[:])
                yield 0

            # RoPE on one 512 slice of one packed tile, in place
            # (quadrant-tiled pswap); cosr/sinr carry the rstd token scale.
            def rope_piece(et, j):
                sl = slice(j * QT, (j + 1) * QT)
                t1 = pw.tile([P, QT], BF16, tag="ropec", name="t1")
                nc.vector.tensor_mul(t1[:], qkT[et][:, sl], cosr[:, sl])
                qks = pw.tile([P, QT], BF16, tag="ropes", name="qks")
                nc.vector.tensor_mul(qks[:], qkT[et][:, sl], sinr[:, sl])
                pr = ps.tile([P, QT], F32, tag="bg", name="pr")
                nc.tensor.matmul(pr[0:DH, :], pswap[0:DH, 0:DH],
                                 qks[0:DH, :], start=True, stop=True,
                                 skip_group_check=True)
                nc.tensor.matmul(pr[DH:P, :], pswap[DH:P, DH:P],
                                 qks[DH:P, :], start=True, stop=True,
                                 skip_group_check=True)
                yield 230
                nc.vector.tensor_add(qkT[et][:, sl], t1[:], pr[:])
                yield 0

            # V projection for one k-chunk, all 4 heads at once.
            def v_chunk(kc):
                pv = ps.tile([P, NH * 65], F32, tag="bg", name="pv")
                for dc in range(DC):
                    nc.tensor.matmul(
                        pv[:],
                        xT[:, dc * n + kc * P:dc * n + (kc + 1) * P],
                        wvp[:, dc * NH * 65:(dc + 1) * NH * 65],
                        start=(dc == 0), stop=(dc == DC - 1),
                        skip_group_check=True)
                vsl = slice(kc * NH * 65, (kc + 1) * NH * 65)
                nc.vector.tensor_scalar_mul(vaug[:, vsl], pv[:],
                                            rstd_p[:, kc:kc + 1])
                nc.gpsimd.memset(vaug[:, kc * NH * 65 + DH::65], 1.0)

            def drain_v01():
                for kc in range(2):
                    vsl = slice(kc * NH * 65, (kc + 1) * NH * 65)
                    nc.vector.tensor_scalar_mul(vaug[:, vsl], pv01[kc][:],
                                                rstd_p[:, kc:kc + 1])
                    nc.gpsimd.memset(vaug[:, kc * NH * 65 + DH::65], 1.0)

            # gates: col-tiled M=4 matmuls, all 4 heads at once; sigmoid as
            # 0.5*tanh(z/2)+0.5 (tanh shares the exp ACT table; bgT holds
            # b_gates/2 host-side).
            def gates_proj():
                pg2 = [ps.tile([DH + NH, QT], F32, tag="bg", name=f"pg{j}")
                       for j in range(2)]
                for dc in range(DC):
                    for qt in range(nqt):
                        rb = (qt % 2) * DH
                        nc.tensor.matmul(
                            pg2[qt // 2][rb:rb + NH, :],
                            wqkg[:, dc * WQ + 512:dc * WQ + 516],
                            xT[:, dc * n + qt * QT:dc * n + (qt + 1) * QT],
                            start=(dc == 0), stop=(dc == DC - 1),
                            skip_group_check=True)
                    yield 250
                for qt in range(nqt):
                    sl = slice(qt * QT, (qt + 1) * QT)
                    rb = (qt % 2) * DH
                    nc.vector.tensor_mul(g4[0:NH, sl],
                                         pg2[qt // 2][rb:rb + NH, :],
                                         rstd_b[rb:rb + NH, sl])
                    yield 0
                nc.scalar.activation(g4[:], g4[:], AF.Tanh, scale=0.5,
                                     bias=bgT[:])
                nc.vector.tensor_scalar(g4[:], g4[:], 0.5, 0.5,
                                        mybir.AluOpType.mult,
                                        mybir.AluOpType.add)
                yield 0
                # scatter head rows into the packed gsh2 tiles via DRAM
                scr4 = pd.tile([NH, n], F32, tag="scr4", name="scr4")
                nc.sync.dma_start(scr4[:], g4[:])
                for h in range(NH):
                    nc.sync.dma_start(gsh(h), scr4[h:h + 1, :])
                yield 0
                if dbg:
                    nc.sync.dma_start(dbg_g4[:], g4[:])
                    yield 0

            # gating of one head pair's finished 512-slice + its out chunks
            def gate_slice(i, qt):
                qsl = slice(qt * QT, (qt + 1) * QT)
                rec = pw.tile([DH + 1, QT], F32, tag="recs", name="rec")
                nc.vector.reciprocal_approx_fast(rec[:], smh2[i][:, qsl])
                yield 0
                for h in (2 * i, 2 * i + 1):
                    rb = (h % 2) * DH
                    ft = pw.tile([1, QT], BF16, tag="fts", name="ft")
                    nc.vector.tensor_mul(ft[:], rec[rb:rb + 1, :],
                                         gsh2[i][rb:rb + 1, qsl])
                    pf = ps.tile([DH, QT], F32, tag="bg", name="pf")
                    nc.tensor.matmul(pf[:], onesrb[0:1, 0:DH], ft[0:1, :],
                                     start=True, stop=True)
                    nc.vector.tensor_mul(oTs[i][rb:rb + DH, qsl],
                                         oTs[i][rb:rb + DH, qsl], pf[:])
                    yield 213

            # one n-chunk of the output projection, DMA'd out when built
            # (DMA queue rotated so the tail chunks transfer in parallel)
            def out_nt(nt):
                ob = pw.tile([P, DIM], BF16, tag="ob", name="ob")
                for dh in range(2):
                    pp2 = ps.tile([P, QT], F32, tag="bg", name="pp2")
                    for ec in range(2):
                        nc.tensor.matmul(
                            pp2[:],
                            oTs[ec][:, nt * P:(nt + 1) * P],
                            wout[:, ec * DIM + dh * QT:
                                 ec * DIM + dh * QT + QT],
                            start=(ec == 0), stop=(ec == 1))
                        yield 216
                    if dh == 0:
                        nc.vector.tensor_copy(ob[:, dh * QT:(dh + 1) * QT],
                                              pp2[:])
                    else:
                        nc.scalar.copy(ob[:, dh * QT:(dh + 1) * QT], pp2[:])
                    yield 0
                # sync/gpsimd only -- a DMA issue on the scalar queue could
                # block the ACT exp stream on an in-flight semaphore wait
                (nc.sync if nt % 2 == 0 else nc.gpsimd).dma_start(
                    out_d[nt * P:(nt + 1) * P, :], ob[:])
                yield 0

            def run_now(gen):
                for _ in gen:
                    pass

            def chain(*gens):
                for g in gens:
                    for c in g:
                        yield c

            # rope on the prologue tiles; v0/v1 drains (wait on rstd_p)
            run_now(rope_piece(0, 0))
            run_now(rope_piece(1, 0))
            run_now(rope_piece(2, 0))
            run_now(rope_piece(3, 0))
            drain_v01()

            if dbg:
                nc.sync.dma_start(dbg_rstd[:], scr[:])

            # ---------------- fused SDPA ----------------
            state = {"credit": 0, "gens": []}

            def drain(budget):
                state["credit"] += budget
                gens = state["gens"]
                while gens and state["credit"] > 0:
                    try:
                        state["credit"] -= next(gens[0])
                    except StopIteration:
                        gens.pop(0)

            def force(gen):
                run_now(gen)

            def pv65(pr, kc, pos, pt):
                voff = pt * 2 * 65
                for e, prs in enumerate((pr[:, 0:QT], pr[:, QT:2 * QT])):
                    nc.tensor.matmul(
                        pos[e][:],
                        vaug[:, kc * NH * 65 + voff + e * 65:
                             kc * NH * 65 + voff + (e + 1) * 65],
                        prs,
                        start=(kc == 0), stop=(kc == nkc - 1),
                        skip_group_check=True)

            # k / q production units: et 2+pt is the k tile, et pt the q tile
            ktile = {(pt, g): chain(qk_proj_tile(2 + pt, g),
                                    rope_piece(2 + pt, g))
                     for pt in range(2) for g in range(1, nqt)}
            qtile = {(pt, j): chain(qk_proj_tile(pt, j), rope_piece(pt, j))
                     for pt in range(2) for j in range(1, nqt)}
            g_gates = gates_proj()

            # 8 segments: (qt, pt) with pt inner.  Background inventory is
            # spread so each segment's ACT exp stream stays dense.
            for qt in range(nqt):
                qsl = slice(qt * QT, (qt + 1) * QT)
                for pt in range(2):
                    seg0 = (qt == 0 and pt == 0)
                    pos = [ps.tile([DH + 1, QT], F32, tag="po", name=f"po{e}")
                           for e in range(2)]

                    if seg0:
                        state["gens"] = [ktile[(0, 1)], ktile[(0, 2)],
                                         ktile[(0, 3)], ktile[(1, 1)],
                                         ktile[(1, 2)], ktile[(1, 3)],
                                         g_gates]
                    elif qt == 0 and pt == 1:
                        state["gens"].append(qtile[(0, 1)])
                        state["gens"].append(qtile[(1, 1)])
                    else:
                        if pt == 0:
                            # pt0 of the previous q-tile was gated one
                            # segment ago (qt >= 2) or now (qt == 1, since
                            # gates land at the end of segment 1)
                            if qt == 1:
                                state["gens"].append(chain(
                                    gate_slice(0, 0), gate_slice(1, 0),
                                    *[out_nt(nt) for nt in range(0, 4)]))
                            else:
                                state["gens"].append(chain(
                                    gate_slice(1, qt - 1),
                                    *[out_nt(nt) for nt in
                                      range(4 * (qt - 1), 4 * (qt - 1) + 4)]))
                        else:
                            # this q-tile's pt0 heads are final: gate them now
                            state["gens"].append(gate_slice(0, qt))
                            if qt + 1 < nqt:
                                state["gens"].append(qtile[(0, qt + 1)])
                                state["gens"].append(qtile[(1, qt + 1)])

                    prev = None
                    for kc in range(nkc):
                        ksl = slice(kc * P, (kc + 1) * P)
                        if qt == 0 and kc >= 4 and kc % 4 == 0:
                            # emission-order deadline: k-tile g fully
                            # emitted before the first scores that read it
                            force(ktile[(pt, kc // 4)])
                        if prev is not None:
                            pv65(*prev, pos, pt)
                        if seg0:
                            if kc + 2 < nkc:
                                v_chunk(kc + 2)
                            drain(200)
                        else:
                            drain(500)
                        sc = ps.tile([P, 2 * QT], F32, tag="sc", name="sc")
                        nc.tensor.matmul(sc[:, 0:QT], qkT[2 + pt][0:DH, ksl],
                                         qkT[pt][0:DH, qsl],
                                         start=True, stop=True,
                                         skip_group_check=True)
                        nc.tensor.matmul(sc[:, QT:2 * QT],
                                         qkT[2 + pt][DH:P, ksl],
                                         qkT[pt][DH:P, qsl],
                                         start=True, stop=True,
                                         skip_group_check=True)
                        pr = pprob.tile([P, 2 * QT], BF16, tag="pr", name="pr")
                        nc.scalar.activation(pr[:], sc[:], AF.Exp,
                                             scale=float(DH) ** -0.5)
                        prev = (pr, kc)
                    pv65(*prev, pos, pt)

                    # epilogue: drain finished pos tiles into oTs/smh
                    for e in range(2):
                        h, rb = 2 * pt + e, e * DH
                        nc.vector.tensor_copy(oTs[pt][rb:rb + DH, qsl],
                                              pos[e][0:DH, :])
                        nc.scalar.copy(smh(h)[0:1, qsl],
                                       pos[e][DH:DH + 1, :])
                    if qt == 0 and pt == 1:
                        # gates fully emitted before gate_slice(., 0)
                        force(g_gates)
                    if pt == 1 and qt + 1 < nqt:
                        # q(qt+1) fully emitted before the next scores
                        force(qtile[(0, qt + 1)])
                        force(qtile[(1, qt + 1)])

            # tail: flush remaining background (includes gate_slice(0, 3)),
            # then the last q-tile's pt1 gating + output projection
            for g in state["gens"]:
                run_now(g)
            state["gens"] = []
            run_now(chain(gate_slice(1, nqt - 1),
                          *[out_nt(nt) for nt in range(4 * (nqt - 1), 4 * nqt)]))

            if dbg:
                nc.sync.dma_start(dbg_qk[:, 0:n], qkT[0][:])
                nc.sync.dma_start(dbg_qk[:, n:2 * n], qkT[1][:])
                nc.sync.dma_start(dbg_qk[:, 2 * n:3 * n], qkT[2][:])
                nc.sync.dma_start(dbg_qk[:, 3 * n:4 * n], qkT[3][:])
                nc.sync.dma_start(dbg_oTs0[:], oTs[0][:])

    nc.compile()
    return nc


def host_prep(x, gamma, w_qkv, w_gates, b_gates, w_out, freqs, n=N):
    """Build the 8 per-core input maps (numpy, host-side)."""
    x = np.asarray(x, dtype=np.float32)
    gamma = np.asarray(gamma, dtype=np.float32)
    w_qkv = np.asarray(w_qkv, dtype=np.float32)
    w_gates = np.asarray(w_gates, dtype=np.float32)
    b_gates = np.asarray(b_gates, dtype=np.float32)
    w_out = np.asarray(w_out, dtype=np.float32)
    freqs = np.asarray(freqs, dtype=np.float32)

    bf = ml_dtypes.bfloat16
    gvec = gamma * (DIM ** 0.5)

    pos = np.arange(n, dtype=np.float32)
    ang = pos[:, None] * freqs[None, :]          # [n, 32]
    idx = (np.arange(P) % DH) // 2               # row -> freq index
    cos_t = np.cos(ang)[:, idx].T.astype(bf)     # [128, n]
    sin_t = np.sin(ang)[:, idx].T.astype(bf)

    PT = np.zeros((DH, DH), dtype=np.float32)
    for i in range(DH // 2):
        PT[2 * i + 1, 2 * i] = -1.0
        PT[2 * i, 2 * i + 1] = 1.0
    pswapT = np.zeros((P, P), dtype=np.float32)
    pswapT[0:DH, 0:DH] = PT
    pswapT[DH:P, DH:P] = PT
    pswapT = pswapT.astype(bf)

    in_maps = []
    for c in range(NCORES):
        bi, hg = divmod(c, 4)
        hs = hg * NH
        xT = np.ascontiguousarray(x[bi, :n].T).astype(bf)
        wq = w_qkv[:, hs * DH:(hs + NH) * DH]
        wk = w_qkv[:, HEADS * DH + hs * DH:HEADS * DH + (hs + NH) * DH]
        wv = w_qkv[:, 2 * HEADS * DH + hs * DH:2 * HEADS * DH + (hs + NH) * DH]
        wg = w_gates[:, hs:hs + NH]
        w_qkg = (np.concatenate([wq, wk, wg], axis=1)
                 * gvec[:, None]).astype(bf)
        w_vp = np.zeros((DIM, NH * 65), dtype=np.float32)
        for h in range(NH):
            w_vp[:, h * 65:h * 65 + DH] = wv[:, h * DH:(h + 1) * DH]
        w_vp = (w_vp * gvec[:, None]).astype(bf)
        w_out_s = w_out[hs * DH:(hs + NH) * DH, :].astype(bf)
        # halved: the kernel computes sigmoid(z+b) as 0.5*tanh((z+b)/2)+0.5
        bgT = (b_gates[hs:hs + NH] / 2.0).reshape(NH, 1).astype(np.float32)
        in_maps.append({
            "xT": xT, "w_qkg": w_qkg, "w_vp": w_vp, "w_out_s": w_out_s,
            "cos_t": cos_t, "sin_t": sin_t, "pswapT": pswapT,
            "bgT": bgT,
        })
    return in_maps


_NC_CACHE = {}


def _ensure_ntff_hook():
    """antenv.axon_hooks is missing on this image; recreate it and register
    the ctypes NTFF profiling hook from trn_agent_boot so trace=True works."""
    try:
        from antenv.axon_hooks import get_axon_ntff_profile_hook  # noqa: F401
        return
    except ImportError:
        pass
    import types
    try:
        import antenv
    except ImportError:
        return
    mod = types.ModuleType("antenv.axon_hooks")
    holder = {}
    mod.set_axon_ntff_profile_hook = lambda h: holder.__setitem__("h", h)
    mod.get_axon_ntff_profile_hook = lambda: holder.get("h")
    sys.modules["antenv.axon_hooks"] = mod
    antenv.axon_hooks = mod
    try:
        from trn_agent_boot.trn_boot import _ntff_profile_via_ctypes
        h = _ntff_profile_via_ctypes("/opt/axon/libaxon_pjrt.so")
        if h is not None:
            mod.set_axon_ntff_profile_hook(h)
    except Exception:
        pass


def run(inputs, trace=False, n=N, dbg=False):
    if trace:
        _ensure_ntff_hook()
    key = (n, dbg)
    if key not in _NC_CACHE:
        _NC_CACHE[key] = build_graph(n, dbg=dbg)
    nc = _NC_CACHE[key]
    in_maps = host_prep(**inputs, n=n)
    kw = {}
    if trace:
        kw = dict(trace=True, trace_cores=[0])
    res = run_bass_kernel_spmd(nc, in_maps, core_ids=list(range(NCORES)), **kw)
    parts = [np.asarray(r["out"], dtype=np.float32) for r in res.results]
    out = np.stack([
        parts[0] + parts[1] + parts[2] + parts[3],
        parts[4] + parts[5] + parts[6] + parts[7],
    ]).astype(np.float32)
    return out, res


def kernel(**inputs):
    out, _ = run(inputs, trace=False)
    return out
